# revision 15
# baseline (speedup 1.0000x reference)
"""Trainium2 Bass kernel for the attention-gate block.

Math (per sample n, after folding BN into the convs):
  X     = x[n, :, ::2, ::2].reshape(C, 4)                 # C=512, L=4
  act_k = relu(Wk' @ X + bk')            k=0,1,2          # D=64 each
  S     = act0^T act1  (4x4);  P = softmax_rows(S)
  Z     = P @ act2^T  (4x64)
  Y     = W4' @ Z^T + b4'                                  # (512, 4)
  out[n,c,h,w] = x[n,c,h,w] + Y[c,h]                       # broadcast over w

Device mapping (per core, 256 samples, blocks of 64):
  - channel packing c = 4p + j (p = partition, j = 0..3): each
    (partition, sample) moves one 256B-contiguous run, so a block is ONE
    big DMA each way (loads on the sync HWDGE queue, stores on scalar's).
    Weights are permuted host-side to match, so compute is unchanged.
  - GEMM1 computes q and k over 4 contraction groups; v is computed
    directly transposed ([samples*4 parts, d]) by swapping matmul
    operands, with its bias folded in via a K=1 ones-row matmul.
  - attention scores for 32 samples at a time come from one [64]x[128,128]
    gram matmul whose block-diagonal 4x4 blocks are the real scores;
    masked exp (ACT + 0/1 block-diag mask on DVE); softmax denominators
    via a ones-column matmul; normalization deferred past the P@V matmul.
  - GEMM2 (BN folded) does the w-broadcast in its rhs AP (step-0 re-read
    of each z column) so the residual add runs on plain stride-1 APs.
"""

import os
import sys

for _p in ("/opt/trn_rl_repo",):
    if _p not in sys.path:
        sys.path.insert(0, _p)

import numpy as np

import concourse.mybir as mybir
from concourse import bacc, tile

EPS = 1e-5
N_TOTAL, C, D, HH, WW = 2048, 512, 64, 4, 4
NCORES = 8
NSH = N_TOTAL // NCORES  # 256 samples per core
BLK = int(os.environ.get("KBLK", "64"))  # samples per block
SUB = 32                 # samples per attention subchunk (4*SUB = 128 cols)
SHIFT = -34.0            # constant exp shift; cancels in the normalization
F32 = mybir.dt.float32

_PROG_CACHE = {}


def build_program(nsh=NSH, blk=BLK, reps=1):
    key = (nsh, blk, reps)
    if key in _PROG_CACHE:
        return _PROG_CACHE[key]

    nc = bacc.Bacc("TRN2", target_bir_lowering=False, debug=False)
    AF = mybir.ActivationFunctionType

    x_in = nc.dram_tensor("x", (nsh, C, HH, WW), F32, kind="ExternalInput")
    wqk = nc.dram_tensor("wqk", (C, 128), F32, kind="ExternalInput")
    bqk = nc.dram_tensor("bqk", (128, 1), F32, kind="ExternalInput")
    w2a = nc.dram_tensor("w2a", (C, D), F32, kind="ExternalInput")
    b2a = nc.dram_tensor("b2a", (1, D), F32, kind="ExternalInput")
    w4t = nc.dram_tensor("w4t", (D, C), F32, kind="ExternalInput")
    b4v = nc.dram_tensor("b4v", (1, C), F32, kind="ExternalInput")
    msk = nc.dram_tensor("msk", (128, 128), F32, kind="ExternalInput")
    out = nc.dram_tensor("out", (nsh, C, HH, WW), F32, kind="ExternalOutput")

    nblk = nsh // blk
    nsub = blk // SUB
    NF = 4 * blk  # free width of a full block of (n, l) columns

    with tile.TileContext(nc) as tc:
        with (
            tc.tile_pool(name="const", bufs=1) as cpool,
            tc.tile_pool(name="xp", bufs=4) as xpool,
            tc.tile_pool(name="work", bufs=4) as wpool,
            tc.tile_pool(name="att", bufs=6) as apool,
            tc.tile_pool(name="ps", bufs=4, space="PSUM") as pspool,
            tc.tile_pool(name="psy", bufs=4, space="PSUM") as pypool,
        ):
            wq_sb = cpool.tile([128, 4, D], F32)
            nc.sync.dma_start(
                wq_sb[:], wqk[:, 0:D].rearrange("(k p) d -> p k d", p=128))
            wk_sb = cpool.tile([128, 4, D], F32)
            nc.sync.dma_start(
                wk_sb[:], wqk[:, D:2 * D].rearrange("(k p) d -> p k d", p=128))
            bq_sb = cpool.tile([D, 1], F32)
            nc.sync.dma_start(bq_sb[:], bqk[0:D])
            bk_sb = cpool.tile([D, 1], F32)
            nc.sync.dma_start(bk_sb[:], bqk[D:2 * D])
            w2a_sb = cpool.tile([128, 4, D], F32)
            nc.sync.dma_start(w2a_sb[:], w2a[:].rearrange("(k p) d -> p k d", p=128))
            b2a_sb = cpool.tile([1, D], F32)
            nc.sync.dma_start(b2a_sb[:], b2a[:])
            w4t_sb = cpool.tile([D, 4, 128], F32)
            nc.sync.dma_start(w4t_sb[:], w4t[:].rearrange("d (k p) -> d k p", p=128))
            b4_sb = cpool.tile([1, C], F32)
            nc.sync.dma_start(b4_sb[:], b4v[:])
            msk_sb = cpool.tile([128, 128], F32)
            nc.sync.dma_start(msk_sb[:], msk[:])
            ones_sb = cpool.tile([1, max(NF, 512)], F32)
            nc.vector.memset(ones_sb[:], 1.0)
            ones_col = cpool.tile([128, 1], F32)
            nc.vector.memset(ones_col[:], 1.0)
            shift_sb = cpool.tile([128, 1], F32)
            nc.vector.memset(shift_sb[:], SHIFT)

            # channel packing c = 4p + j: one DMA per block each way
            xv = x_in[:].rearrange("(b n) (p j) h w -> b p n (j h w)", j=4, n=blk)
            ov = out[:].rearrange("(b n) (p j) h w -> b p n (j h w)", j=4, n=blk)

            for b in [b for _ in range(reps) for b in range(nblk)]:
                x_t = xpool.tile([128, blk, 64], F32, tag="x")
                nc.sync.dma_start(x_t[:], xv[b])
                xtv = x_t[:].rearrange("p n (j h w) -> p n j h w", j=4, h=4)

                # gather the ::2,::2 columns -> [128, j, n, l] with l=(h',w')
                xr = wpool.tile([128, 4, blk, 4], F32, tag="xr")
                for j in range(4):
                    nc.vector.tensor_copy(
                        xr[:, j].rearrange("p n (a c) -> p n a c", a=2),
                        xtv[:, :, j, 0:4:2, 0:4:2],
                    )
                xrf = xr[:].rearrange("p j n l -> p j (n l)")

                # GEMM1 q and k: [c=512 contraction] -> psum [64, NF] each
                ps_q = pspool.tile([D, NF], F32, tag="ps")
                ps_k = pspool.tile([D, NF], F32, tag="ps")
                for j in range(4):
                    nc.tensor.matmul(
                        ps_q[:], lhsT=wq_sb[:, j], rhs=xrf[:, j],
                        start=(j == 0), stop=(j == 3),
                    )
                for j in range(4):
                    nc.tensor.matmul(
                        ps_k[:], lhsT=wk_sb[:, j], rhs=xrf[:, j],
                        start=(j == 0), stop=(j == 3),
                    )
                a_q = wpool.tile([D, NF], F32, tag="aq")
                nc.scalar.activation(a_q[:], ps_q[:], AF.Relu, bias=bq_sb[:])
                a_k = wpool.tile([D, NF], F32, tag="ak")
                nc.scalar.activation(a_k[:], ps_k[:], AF.Relu, bias=bk_sb[:])

                z_subs = []
                for s in range(nsub):
                    cl = slice(s * 128, s * 128 + 128)
                    # v, transposed: [(n,m)=128 parts, d]
                    ps_vt = pspool.tile([128, D], F32, tag="ps")
                    for j in range(4):
                        nc.tensor.matmul(
                            ps_vt[:], lhsT=xrf[:, j, cl], rhs=w2a_sb[:, j],
                            start=(j == 0), stop=False,
                        )
                    nc.tensor.matmul(
                        ps_vt[:], lhsT=ones_sb[:, 0:128], rhs=b2a_sb[:],
                        start=False, stop=True,
                    )
                    a2t = apool.tile([128, D], F32, tag="a2t")
                    nc.scalar.activation(a2t[:], ps_vt[:], AF.Relu)

                    # gram: [(n,m) parts, (n,l) cols]; diag 4x4 blocks = S^T
                    ps_g = pspool.tile([128, 128], F32, tag="ps")
                    nc.tensor.matmul(
                        ps_g[:], lhsT=a_k[:, cl], rhs=a_q[:, cl],
                        start=True, stop=True,
                    )
                    e_t = apool.tile([128, 128], F32, tag="e")
                    nc.scalar.activation(e_t[:], ps_g[:], AF.Exp, bias=shift_sb[:])
                    p0 = apool.tile([128, 128], F32, tag="p0")
                    nc.vector.tensor_mul(p0[:], e_t[:], msk_sb[:])

                    # Z^T: [d=64, (n,l)=128]; denominators via ones column
                    ps_z = pspool.tile([D, 128], F32, tag="ps")
                    nc.tensor.matmul(
                        ps_z[:], lhsT=a2t[:], rhs=p0[:], start=True, stop=True,
                    )
                    ps_d = pspool.tile([1, 128], F32, tag="ps")
                    nc.tensor.matmul(
                        ps_d[:], lhsT=ones_col[:], rhs=p0[:],
                        start=True, stop=True,
                    )
                    r_sb = apool.tile([1, 128], F32, tag="r")
                    nc.vector.reciprocal(r_sb[:], ps_d[:])
                    ps_r = pspool.tile([D, 128], F32, tag="ps")
                    nc.tensor.matmul(
                        ps_r[:], lhsT=ones_sb[:, 0:D], rhs=r_sb[:],
                        start=True, stop=True,
                    )
                    r64_sb = apool.tile([D, 128], F32, tag="r64")
                    nc.scalar.activation(r64_sb[:], ps_r[:], AF.Copy)
                    z_t = apool.tile([D, 4 * SUB], F32, tag="z")
                    nc.vector.tensor_mul(z_t[:], ps_z[:], r64_sb[:])
                    z_subs.append(z_t)

                # GEMM2 + bias; the w-broadcast happens in the matmul rhs
                # (step-0 AP re-reads each z column 4x) so the residual
                # add runs on plain stride-1 APs at full DVE rate.
                nsb = SUB
                for j in range(4):
                    for h in range(nsub):
                        nsl = slice(h * nsb, (h + 1) * nsb)
                        zv = (
                            z_subs[h][:]
                            .rearrange("p (n l) -> p n l", l=4)
                            .unsqueeze(3)
                            .broadcast_to((D, nsb, 4, 4))
                        )
                        ps_y = pypool.tile([128, 16 * nsb], F32, tag="psy")
                        nc.tensor.matmul(
                            ps_y[:], lhsT=w4t_sb[:, j], rhs=zv[:],
                            start=True, stop=False,
                        )
                        nc.tensor.matmul(
                            ps_y[:], lhsT=b4_sb[:, j * 128:(j + 1) * 128],
                            rhs=ones_sb[:, 0:16 * nsb], start=False, stop=True,
                        )
                        nc.vector.tensor_add(
                            xtv[:, nsl, j], xtv[:, nsl, j],
                            ps_y[:].rearrange("p (n h w) -> p n h w", h=4, w=4),
                        )

                # store on the scalar HWDGE queue to overlap with loads
                nc.scalar.dma_start(ov[b], x_t[:])

    nc.compile()
    _PROG_CACHE[key] = nc
    return nc


def prep_params(W123, b123, g123, be123, m123, v123, W4, b4, g4, be4, m4, v4):
    """Fold BN into the convs; permute channels for the c=4p+j packing."""
    f32 = np.float32
    s123 = (g123 / np.sqrt(v123 + EPS)).astype(f32)            # (3, D)
    Wf = (W123 * s123[:, :, None]).astype(f32)                 # (3, D, C)
    bf = ((b123 - m123) * s123 + be123).astype(f32)            # (3, D)
    s4 = (g4 / np.sqrt(v4 + EPS)).astype(f32)                  # (C,)
    W4f = (W4 * s4[:, None]).astype(f32)                       # (C, D)
    b4f = ((b4 - m4) * s4 + be4).astype(f32)                   # (C,)

    # perm[j*128 + p] = 4p + j : row j*128+p of a device weight tensor
    # holds original channel 4p+j (matching the x packing).
    p_idx, j_idx = np.meshgrid(np.arange(128), np.arange(4), indexing="ij")
    perm = (4 * p_idx + j_idx).T.reshape(-1)                   # (512,)

    wqk = np.concatenate([Wf[0].T, Wf[1].T], axis=1)[perm]     # (C, 128)
    bqk = np.concatenate([bf[0], bf[1]])[:, None]              # (128, 1)
    w2a = np.ascontiguousarray(Wf[2].T[perm])                  # (C, D)
    b2a = bf[2][None, :]                                       # (1, D)
    w4t = np.ascontiguousarray(W4f.T[:, perm])                 # (D, C)
    b4v = b4f[perm][None, :]                                   # (1, C)
    msk = np.kron(np.eye(SUB, dtype=f32), np.ones((4, 4), f32))  # (128, 128)
    return dict(
        wqk=np.ascontiguousarray(wqk), bqk=np.ascontiguousarray(bqk),
        w2a=w2a, b2a=np.ascontiguousarray(b2a),
        w4t=w4t, b4v=np.ascontiguousarray(b4v), msk=msk,
    )


def _run(inputs, trace=False, **spmd_kwargs):
    from concourse.bass_utils import run_bass_kernel_spmd

    x = np.ascontiguousarray(np.asarray(inputs["x"], dtype=np.float32))
    params = prep_params(**{k: np.asarray(v, np.float64)
                            for k, v in inputs.items() if k != "x"})
    nc = build_program()
    in_maps = [
        {"x": x[i * NSH:(i + 1) * NSH], **params} for i in range(NCORES)
    ]
    res = run_bass_kernel_spmd(
        nc, in_maps, list(range(NCORES)), trace=trace, **spmd_kwargs
    )
    outs = np.concatenate(
        [np.asarray(res.results[i]["out"]) for i in range(NCORES)], axis=0
    )
    return outs, res


def kernel(**inputs):
    outs, _ = _run(inputs)
    return outs


# revision 16
# speedup vs baseline: 1.2447x; 1.2447x over previous
"""Trainium2 Bass kernel for the attention-gate block.

Math (per sample n, after folding BN into the convs):
  X     = x[n, :, ::2, ::2].reshape(C, 4)                 # C=512, L=4
  act_k = relu(Wk' @ X + bk')            k=0,1,2          # D=64 each
  S     = act0^T act1  (4x4);  P = softmax_rows(S)
  Z     = P @ act2^T  (4x64)
  Y     = W4' @ Z^T + b4'                                  # (512, 4)
  out[n,c,h,w] = x[n,c,h,w] + Y[c,h]                       # broadcast over w

Device mapping (per core, 256 samples, blocks of 64):
  - channel packing c = 4p + j (p = partition, j = 0..3): each
    (partition, sample) moves one 256B-contiguous run, so a block is ONE
    big DMA each way (loads on the sync HWDGE queue, stores on scalar's).
    Weights are permuted host-side to match, so compute is unchanged.
  - GEMM1 computes q and k over 4 contraction groups; v is computed
    directly transposed ([samples*4 parts, d]) by swapping matmul
    operands, with its bias folded in via a K=1 ones-row matmul.
  - attention scores for 32 samples at a time come from one [64]x[128,128]
    gram matmul whose block-diagonal 4x4 blocks are the real scores;
    masked exp (ACT + 0/1 block-diag mask on DVE); softmax denominators
    via a ones-column matmul; normalization deferred past the P@V matmul.
  - GEMM2 (BN folded) does the w-broadcast in its rhs AP (step-0 re-read
    of each z column) so the residual add runs on plain stride-1 APs.
"""

import os
import sys

for _p in ("/opt/trn_rl_repo",):
    if _p not in sys.path:
        sys.path.insert(0, _p)

import numpy as np

import concourse.mybir as mybir
from concourse import bacc, tile

EPS = 1e-5
N_TOTAL, C, D, HH, WW = 2048, 512, 64, 4, 4
NCORES = 8
NSH = N_TOTAL // NCORES  # 256 samples per core
BLK = int(os.environ.get("KBLK", "64"))  # samples per block
SUB = 32                 # samples per attention subchunk (4*SUB = 128 cols)
SHIFT = -34.0            # constant exp shift; cancels in the normalization
F32 = mybir.dt.float32

_PROG_CACHE = {}


def build_program(nsh=NSH, blk=BLK, reps=1):
    key = (nsh, blk, reps)
    if key in _PROG_CACHE:
        return _PROG_CACHE[key]

    nc = bacc.Bacc("TRN2", target_bir_lowering=False, debug=False)
    AF = mybir.ActivationFunctionType

    x_in = nc.dram_tensor("x", (nsh, C, HH, WW), F32, kind="ExternalInput")
    wqk = nc.dram_tensor("wqk", (C, 128), F32, kind="ExternalInput")
    bqk = nc.dram_tensor("bqk", (128, 1), F32, kind="ExternalInput")
    w2a = nc.dram_tensor("w2a", (C, D), F32, kind="ExternalInput")
    b2a = nc.dram_tensor("b2a", (1, D), F32, kind="ExternalInput")
    w4t = nc.dram_tensor("w4t", (D, C), F32, kind="ExternalInput")
    b4v = nc.dram_tensor("b4v", (1, C), F32, kind="ExternalInput")
    msk = nc.dram_tensor("msk", (128, 128), F32, kind="ExternalInput")
    out = nc.dram_tensor("out", (nsh, C, HH, WW), F32, kind="ExternalOutput")

    nblk = nsh // blk
    nsub = blk // SUB
    NF = 4 * blk  # free width of a full block of (n, l) columns

    with tile.TileContext(nc) as tc:
        with (
            tc.tile_pool(name="const", bufs=1) as cpool,
            tc.tile_pool(name="xp", bufs=4) as xpool,
            tc.tile_pool(name="work", bufs=4) as wpool,
            tc.tile_pool(name="att", bufs=6) as apool,
            tc.tile_pool(name="ps", bufs=6, space="PSUM") as pspool,
            tc.tile_pool(name="psy", bufs=2, space="PSUM") as pypool,
        ):
            wq_sb = cpool.tile([128, 4, D], F32)
            nc.sync.dma_start(
                wq_sb[:], wqk[:, 0:D].rearrange("(k p) d -> p k d", p=128))
            wk_sb = cpool.tile([128, 4, D], F32)
            nc.sync.dma_start(
                wk_sb[:], wqk[:, D:2 * D].rearrange("(k p) d -> p k d", p=128))
            bq_sb = cpool.tile([D, 1], F32)
            nc.sync.dma_start(bq_sb[:], bqk[0:D])
            bk_sb = cpool.tile([D, 1], F32)
            nc.sync.dma_start(bk_sb[:], bqk[D:2 * D])
            w2a_sb = cpool.tile([128, 4, D], F32)
            nc.sync.dma_start(w2a_sb[:], w2a[:].rearrange("(k p) d -> p k d", p=128))
            b2a_sb = cpool.tile([1, D], F32)
            nc.sync.dma_start(b2a_sb[:], b2a[:])
            w4t_sb = cpool.tile([D, 4, 128], F32)
            nc.sync.dma_start(w4t_sb[:], w4t[:].rearrange("d (k p) -> d k p", p=128))
            b4_sb = cpool.tile([1, C], F32)
            nc.sync.dma_start(b4_sb[:], b4v[:])
            msk_sb = cpool.tile([128, 128], F32)
            nc.sync.dma_start(msk_sb[:], msk[:])
            ones_sb = cpool.tile([1, max(NF, 512)], F32)
            nc.vector.memset(ones_sb[:], 1.0)
            ones_col = cpool.tile([128, 1], F32)
            nc.vector.memset(ones_col[:], 1.0)
            shift_sb = cpool.tile([128, 1], F32)
            nc.vector.memset(shift_sb[:], SHIFT)

            # channel packing c = 4p + j: one DMA per block each way
            xv = x_in[:].rearrange("(b n) (p j) h w -> b p n (j h w)", j=4, n=blk)
            ov = out[:].rearrange("(b n) (p j) h w -> b p n (j h w)", j=4, n=blk)

            for b in [b for _ in range(reps) for b in range(nblk)]:
                x_t = xpool.tile([128, blk, 64], F32, tag="x")
                nc.sync.dma_start(x_t[:], xv[b])
                xtv = x_t[:].rearrange("p n (j h w) -> p n j h w", j=4, h=4)

                # gather the ::2,::2 columns -> [128, j, n, l] with l=(h',w')
                xr = wpool.tile([128, 4, blk, 4], F32, tag="xr")
                for j in range(4):
                    nc.vector.tensor_copy(
                        xr[:, j].rearrange("p n (a c) -> p n a c", a=2),
                        xtv[:, :, j, 0:4:2, 0:4:2],
                    )
                xrf = xr[:].rearrange("p j n l -> p j (n l)")

                # GEMM1 q and k: [c=512 contraction] -> psum [64, NF] each
                ps_q = pspool.tile([D, NF], F32, tag="ps")
                ps_k = pspool.tile([D, NF], F32, tag="ps")
                for j in range(4):
                    nc.tensor.matmul(
                        ps_q[:], lhsT=wq_sb[:, j], rhs=xrf[:, j],
                        start=(j == 0), stop=(j == 3),
                    )
                for j in range(4):
                    nc.tensor.matmul(
                        ps_k[:], lhsT=wk_sb[:, j], rhs=xrf[:, j],
                        start=(j == 0), stop=(j == 3),
                    )
                a_q = wpool.tile([D, NF], F32, tag="aq")
                nc.scalar.activation(a_q[:], ps_q[:], AF.Relu, bias=bq_sb[:])
                a_k = wpool.tile([D, NF], F32, tag="ak")
                nc.scalar.activation(a_k[:], ps_k[:], AF.Relu, bias=bk_sb[:])

                # phase 1: independent PE work for all subchunks
                ph_vt, ph_g = [], []
                for s in range(nsub):
                    cl = slice(s * 128, s * 128 + 128)
                    ps_vt = pspool.tile([128, D], F32, tag="ps")
                    for j in range(4):
                        nc.tensor.matmul(
                            ps_vt[:], lhsT=xrf[:, j, cl], rhs=w2a_sb[:, j],
                            start=(j == 0), stop=False,
                        )
                    nc.tensor.matmul(
                        ps_vt[:], lhsT=ones_sb[:, 0:128], rhs=b2a_sb[:],
                        start=False, stop=True,
                    )
                    ps_g = pspool.tile([128, 128], F32, tag="ps")
                    nc.tensor.matmul(
                        ps_g[:], lhsT=a_k[:, cl], rhs=a_q[:, cl],
                        start=True, stop=True,
                    )
                    ph_vt.append(ps_vt)
                    ph_g.append(ps_g)
                # phase 2: ACT/DVE consumers for all subchunks
                ph_a2t, ph_p0 = [], []
                for s in range(nsub):
                    a2t = apool.tile([128, D], F32, tag="a2t")
                    nc.scalar.activation(a2t[:], ph_vt[s][:], AF.Relu)
                    e_t = apool.tile([128, 128], F32, tag="e")
                    nc.scalar.activation(e_t[:], ph_g[s][:], AF.Exp,
                                         bias=shift_sb[:])
                    p0 = apool.tile([128, 128], F32, tag="p0")
                    nc.vector.tensor_mul(p0[:], e_t[:], msk_sb[:])
                    ph_a2t.append(a2t)
                    ph_p0.append(p0)
                # phase 3: dependent matmuls + normalization per subchunk
                z_subs = []
                for s in range(nsub):
                    a2t, p0 = ph_a2t[s], ph_p0[s]
                    ps_z = pspool.tile([D, 128], F32, tag="ps")
                    nc.tensor.matmul(
                        ps_z[:], lhsT=a2t[:], rhs=p0[:], start=True, stop=True,
                    )
                    ps_d = pspool.tile([1, 128], F32, tag="ps")
                    nc.tensor.matmul(
                        ps_d[:], lhsT=ones_col[:], rhs=p0[:],
                        start=True, stop=True,
                    )
                    r_sb = apool.tile([1, 128], F32, tag="r")
                    nc.vector.reciprocal(r_sb[:], ps_d[:])
                    ps_r = pspool.tile([D, 128], F32, tag="ps")
                    nc.tensor.matmul(
                        ps_r[:], lhsT=ones_sb[:, 0:D], rhs=r_sb[:],
                        start=True, stop=True,
                    )
                    r64_sb = apool.tile([D, 128], F32, tag="r64")
                    nc.scalar.activation(r64_sb[:], ps_r[:], AF.Copy)
                    z_t = apool.tile([D, 4 * SUB], F32, tag="z")
                    nc.vector.tensor_mul(z_t[:], ps_z[:], r64_sb[:])
                    z_subs.append(z_t)

                # GEMM2 + bias; the w-broadcast happens in the matmul rhs
                # (step-0 AP re-reads each z column 4x) so the residual
                # add runs on plain stride-1 APs at full DVE rate.
                nsb = SUB
                for j in range(4):
                    for h in range(nsub):
                        nsl = slice(h * nsb, (h + 1) * nsb)
                        zv = (
                            z_subs[h][:]
                            .rearrange("p (n l) -> p n l", l=4)
                            .unsqueeze(3)
                            .broadcast_to((D, nsb, 4, 4))
                        )
                        ps_y = pypool.tile([128, 16 * nsb], F32, tag="psy")
                        nc.tensor.matmul(
                            ps_y[:], lhsT=w4t_sb[:, j], rhs=zv[:],
                            start=True, stop=False,
                        )
                        nc.tensor.matmul(
                            ps_y[:], lhsT=b4_sb[:, j * 128:(j + 1) * 128],
                            rhs=ones_sb[:, 0:16 * nsb], start=False, stop=True,
                        )
                        nc.vector.tensor_add(
                            xtv[:, nsl, j], xtv[:, nsl, j],
                            ps_y[:].rearrange("p (n h w) -> p n h w", h=4, w=4),
                        )

                # store on the scalar HWDGE queue to overlap with loads
                nc.scalar.dma_start(ov[b], x_t[:])

    nc.compile()
    _PROG_CACHE[key] = nc
    return nc


def prep_params(W123, b123, g123, be123, m123, v123, W4, b4, g4, be4, m4, v4):
    """Fold BN into the convs; permute channels for the c=4p+j packing."""
    f32 = np.float32
    s123 = (g123 / np.sqrt(v123 + EPS)).astype(f32)            # (3, D)
    Wf = (W123 * s123[:, :, None]).astype(f32)                 # (3, D, C)
    bf = ((b123 - m123) * s123 + be123).astype(f32)            # (3, D)
    s4 = (g4 / np.sqrt(v4 + EPS)).astype(f32)                  # (C,)
    W4f = (W4 * s4[:, None]).astype(f32)                       # (C, D)
    b4f = ((b4 - m4) * s4 + be4).astype(f32)                   # (C,)

    # perm[j*128 + p] = 4p + j : row j*128+p of a device weight tensor
    # holds original channel 4p+j (matching the x packing).
    p_idx, j_idx = np.meshgrid(np.arange(128), np.arange(4), indexing="ij")
    perm = (4 * p_idx + j_idx).T.reshape(-1)                   # (512,)

    wqk = np.concatenate([Wf[0].T, Wf[1].T], axis=1)[perm]     # (C, 128)
    bqk = np.concatenate([bf[0], bf[1]])[:, None]              # (128, 1)
    w2a = np.ascontiguousarray(Wf[2].T[perm])                  # (C, D)
    b2a = bf[2][None, :]                                       # (1, D)
    w4t = np.ascontiguousarray(W4f.T[:, perm])                 # (D, C)
    b4v = b4f[perm][None, :]                                   # (1, C)
    msk = np.kron(np.eye(SUB, dtype=f32), np.ones((4, 4), f32))  # (128, 128)
    return dict(
        wqk=np.ascontiguousarray(wqk), bqk=np.ascontiguousarray(bqk),
        w2a=w2a, b2a=np.ascontiguousarray(b2a),
        w4t=w4t, b4v=np.ascontiguousarray(b4v), msk=msk,
    )


def _run(inputs, trace=False, **spmd_kwargs):
    from concourse.bass_utils import run_bass_kernel_spmd

    x = np.ascontiguousarray(np.asarray(inputs["x"], dtype=np.float32))
    params = prep_params(**{k: np.asarray(v, np.float64)
                            for k, v in inputs.items() if k != "x"})
    nc = build_program()
    in_maps = [
        {"x": x[i * NSH:(i + 1) * NSH], **params} for i in range(NCORES)
    ]
    res = run_bass_kernel_spmd(
        nc, in_maps, list(range(NCORES)), trace=trace, **spmd_kwargs
    )
    outs = np.concatenate(
        [np.asarray(res.results[i]["out"]) for i in range(NCORES)], axis=0
    )
    return outs, res


def kernel(**inputs):
    outs, _ = _run(inputs)
    return outs


# revision 17
# speedup vs baseline: 1.2493x; 1.0037x over previous
"""Trainium2 Bass kernel for the attention-gate block.

Math (per sample n, after folding BN into the convs):
  X     = x[n, :, ::2, ::2].reshape(C, 4)                 # C=512, L=4
  act_k = relu(Wk' @ X + bk')            k=0,1,2          # D=64 each
  S     = act0^T act1  (4x4);  P = softmax_rows(S)
  Z     = P @ act2^T  (4x64)
  Y     = W4' @ Z^T + b4'                                  # (512, 4)
  out[n,c,h,w] = x[n,c,h,w] + Y[c,h]                       # broadcast over w

Device mapping (per core, 256 samples, blocks of 64):
  - channel packing c = 4p + j (p = partition, j = 0..3): each
    (partition, sample) moves one 256B-contiguous run, so a block is ONE
    big DMA each way (loads on the sync HWDGE queue, stores on scalar's).
    Weights are permuted host-side to match, so compute is unchanged.
  - GEMM1 computes q and k over 4 contraction groups; v is computed
    directly transposed ([samples*4 parts, d]) by swapping matmul
    operands, with its bias folded in via a K=1 ones-row matmul.
  - attention scores for 32 samples at a time come from one [64]x[128,128]
    gram matmul whose block-diagonal 4x4 blocks are the real scores;
    masked exp (ACT + 0/1 block-diag mask on DVE); softmax denominators
    via a ones-column matmul; normalization deferred past the P@V matmul.
  - GEMM2 (BN folded) does the w-broadcast in its rhs AP (step-0 re-read
    of each z column) so the residual add runs on plain stride-1 APs.
"""

import os
import sys

for _p in ("/opt/trn_rl_repo",):
    if _p not in sys.path:
        sys.path.insert(0, _p)

import numpy as np

import concourse.mybir as mybir
from concourse import bacc, tile

EPS = 1e-5
N_TOTAL, C, D, HH, WW = 2048, 512, 64, 4, 4
NCORES = 8
NSH = N_TOTAL // NCORES  # 256 samples per core
BLK = int(os.environ.get("KBLK", "64"))  # samples per block
SUB = 32                 # samples per attention subchunk (4*SUB = 128 cols)
SHIFT = -34.0            # constant exp shift; cancels in the normalization
F32 = mybir.dt.float32

_PROG_CACHE = {}


def build_program(nsh=NSH, blk=BLK, reps=1):
    key = (nsh, blk, reps)
    if key in _PROG_CACHE:
        return _PROG_CACHE[key]

    nc = bacc.Bacc("TRN2", target_bir_lowering=False, debug=False)
    AF = mybir.ActivationFunctionType

    x_in = nc.dram_tensor("x", (nsh, C, HH, WW), F32, kind="ExternalInput")
    wqk = nc.dram_tensor("wqk", (C, 128), F32, kind="ExternalInput")
    bqk = nc.dram_tensor("bqk", (128, 1), F32, kind="ExternalInput")
    w2a = nc.dram_tensor("w2a", (C, D), F32, kind="ExternalInput")
    b2a = nc.dram_tensor("b2a", (1, D), F32, kind="ExternalInput")
    w4t = nc.dram_tensor("w4t", (D, C), F32, kind="ExternalInput")
    b4v = nc.dram_tensor("b4v", (1, C), F32, kind="ExternalInput")
    msk = nc.dram_tensor("msk", (128, 128), F32, kind="ExternalInput")
    out = nc.dram_tensor("out", (nsh, C, HH, WW), F32, kind="ExternalOutput")

    nblk = nsh // blk
    nsub = blk // SUB
    NF = 4 * blk  # free width of a full block of (n, l) columns

    with tile.TileContext(nc) as tc:
        with (
            tc.tile_pool(name="const", bufs=1) as cpool,
            tc.tile_pool(name="xp", bufs=(3 if blk >= 128 else 4)) as xpool,
            tc.tile_pool(name="work", bufs=4) as wpool,
            tc.tile_pool(name="att", bufs=6) as apool,
            tc.tile_pool(name="ps", bufs=6, space="PSUM") as pspool,
            tc.tile_pool(name="psy", bufs=2, space="PSUM") as pypool,
        ):
            wq_sb = cpool.tile([128, 4, D], F32)
            nc.sync.dma_start(
                wq_sb[:], wqk[:, 0:D].rearrange("(k p) d -> p k d", p=128))
            wk_sb = cpool.tile([128, 4, D], F32)
            nc.sync.dma_start(
                wk_sb[:], wqk[:, D:2 * D].rearrange("(k p) d -> p k d", p=128))
            bq_sb = cpool.tile([D, 1], F32)
            nc.sync.dma_start(bq_sb[:], bqk[0:D])
            bk_sb = cpool.tile([D, 1], F32)
            nc.sync.dma_start(bk_sb[:], bqk[D:2 * D])
            w2a_sb = cpool.tile([128, 4, D], F32)
            nc.sync.dma_start(w2a_sb[:], w2a[:].rearrange("(k p) d -> p k d", p=128))
            b2a_sb = cpool.tile([1, D], F32)
            nc.sync.dma_start(b2a_sb[:], b2a[:])
            w4t_sb = cpool.tile([D, 4, 128], F32)
            nc.sync.dma_start(w4t_sb[:], w4t[:].rearrange("d (k p) -> d k p", p=128))
            b4_sb = cpool.tile([1, C], F32)
            nc.sync.dma_start(b4_sb[:], b4v[:])
            msk_sb = cpool.tile([128, 128], F32)
            nc.sync.dma_start(msk_sb[:], msk[:])
            ones_sb = cpool.tile([1, max(NF, 512)], F32)
            nc.vector.memset(ones_sb[:], 1.0)
            ones_col = cpool.tile([128, 1], F32)
            nc.vector.memset(ones_col[:], 1.0)
            shift_sb = cpool.tile([128, 1], F32)
            nc.vector.memset(shift_sb[:], SHIFT)

            # channel packing c = 4p + j: one DMA per block each way
            xv = x_in[:].rearrange("(b n) (p j) h w -> b p n (j h w)", j=4, n=blk)
            ov = out[:].rearrange("(b n) (p j) h w -> b p n (j h w)", j=4, n=blk)

            for b in [b for _ in range(reps) for b in range(nblk)]:
                x_t = xpool.tile([128, blk, 64], F32, tag="x")
                nc.sync.dma_start(x_t[:], xv[b])
                xtv = x_t[:].rearrange("p n (j h w) -> p n j h w", j=4, h=4)

                # gather the ::2,::2 columns -> [128, j, n, l] with l=(h',w')
                xr = wpool.tile([128, 4, blk, 4], F32, tag="xr")
                for j in range(4):
                    nc.vector.tensor_copy(
                        xr[:, j].rearrange("p n (a c) -> p n a c", a=2),
                        xtv[:, :, j, 0:4:2, 0:4:2],
                    )
                xrf = xr[:].rearrange("p j n l -> p j (n l)")

                # GEMM1 q and k: [c=512 contraction] -> psum [64, NF] each
                ps_q = pspool.tile([D, NF], F32, tag="ps")
                ps_k = pspool.tile([D, NF], F32, tag="ps")
                for j in range(4):
                    nc.tensor.matmul(
                        ps_q[:], lhsT=wq_sb[:, j], rhs=xrf[:, j],
                        start=(j == 0), stop=(j == 3),
                    )
                for j in range(4):
                    nc.tensor.matmul(
                        ps_k[:], lhsT=wk_sb[:, j], rhs=xrf[:, j],
                        start=(j == 0), stop=(j == 3),
                    )
                a_q = wpool.tile([D, NF], F32, tag="aq")
                nc.scalar.activation(a_q[:], ps_q[:], AF.Relu, bias=bq_sb[:])
                a_k = wpool.tile([D, NF], F32, tag="ak")
                nc.scalar.activation(a_k[:], ps_k[:], AF.Relu, bias=bk_sb[:])

                # phase 1: independent PE work for all subchunks
                ph_vt, ph_g = [], []
                for s in range(nsub):
                    cl = slice(s * 128, s * 128 + 128)
                    ps_vt = pspool.tile([128, D], F32, tag="ps")
                    for j in range(4):
                        nc.tensor.matmul(
                            ps_vt[:], lhsT=xrf[:, j, cl], rhs=w2a_sb[:, j],
                            start=(j == 0), stop=False,
                        )
                    nc.tensor.matmul(
                        ps_vt[:], lhsT=ones_sb[:, 0:128], rhs=b2a_sb[:],
                        start=False, stop=True,
                    )
                    ps_g = pspool.tile([128, 128], F32, tag="ps")
                    nc.tensor.matmul(
                        ps_g[:], lhsT=a_k[:, cl], rhs=a_q[:, cl],
                        start=True, stop=True,
                    )
                    ph_vt.append(ps_vt)
                    ph_g.append(ps_g)
                # phase 2: ACT/DVE consumers for all subchunks
                ph_a2t, ph_p0 = [], []
                for s in range(nsub):
                    a2t = apool.tile([128, D], F32, tag="a2t")
                    nc.scalar.activation(a2t[:], ph_vt[s][:], AF.Relu)
                    e_t = apool.tile([128, 128], F32, tag="e")
                    nc.scalar.activation(e_t[:], ph_g[s][:], AF.Exp,
                                         bias=shift_sb[:])
                    p0 = apool.tile([128, 128], F32, tag="p0")
                    nc.vector.tensor_mul(p0[:], e_t[:], msk_sb[:])
                    ph_a2t.append(a2t)
                    ph_p0.append(p0)
                # phase 3: dependent matmuls + normalization per subchunk
                z_subs = []
                for s in range(nsub):
                    a2t, p0 = ph_a2t[s], ph_p0[s]
                    ps_z = pspool.tile([D, 128], F32, tag="ps")
                    nc.tensor.matmul(
                        ps_z[:], lhsT=a2t[:], rhs=p0[:], start=True, stop=True,
                    )
                    ps_d = pspool.tile([1, 128], F32, tag="ps")
                    nc.tensor.matmul(
                        ps_d[:], lhsT=ones_col[:], rhs=p0[:],
                        start=True, stop=True,
                    )
                    r_sb = apool.tile([1, 128], F32, tag="r")
                    nc.vector.reciprocal(r_sb[:], ps_d[:])
                    ps_r = pspool.tile([D, 128], F32, tag="ps")
                    nc.tensor.matmul(
                        ps_r[:], lhsT=ones_sb[:, 0:D], rhs=r_sb[:],
                        start=True, stop=True,
                    )
                    r64_sb = apool.tile([D, 128], F32, tag="r64")
                    nc.scalar.activation(r64_sb[:], ps_r[:], AF.Copy)
                    z_t = apool.tile([D, 4 * SUB], F32, tag="z")
                    nc.vector.tensor_mul(z_t[:], ps_z[:], r64_sb[:])
                    z_subs.append(z_t)

                # GEMM2 + bias; the w-broadcast happens in the matmul rhs
                # (step-0 AP re-reads each z column 4x) so the residual
                # add runs on plain stride-1 APs at full DVE rate.
                nsb = SUB
                for j in range(4):
                    for h in range(nsub):
                        nsl = slice(h * nsb, (h + 1) * nsb)
                        zv = (
                            z_subs[h][:]
                            .rearrange("p (n l) -> p n l", l=4)
                            .unsqueeze(3)
                            .broadcast_to((D, nsb, 4, 4))
                        )
                        ps_y = pypool.tile([128, 16 * nsb], F32, tag="psy")
                        nc.tensor.matmul(
                            ps_y[:], lhsT=w4t_sb[:, j], rhs=zv[:],
                            start=True, stop=False,
                        )
                        nc.tensor.matmul(
                            ps_y[:], lhsT=b4_sb[:, j * 128:(j + 1) * 128],
                            rhs=ones_sb[:, 0:16 * nsb], start=False, stop=True,
                        )
                        nc.vector.tensor_add(
                            xtv[:, nsl, j], xtv[:, nsl, j],
                            ps_y[:].rearrange("p (n h w) -> p n h w", h=4, w=4),
                        )

                # store on the scalar HWDGE queue to overlap with loads
                nc.scalar.dma_start(ov[b], x_t[:])

    nc.compile()
    _PROG_CACHE[key] = nc
    return nc


def prep_params(W123, b123, g123, be123, m123, v123, W4, b4, g4, be4, m4, v4):
    """Fold BN into the convs; permute channels for the c=4p+j packing."""
    f32 = np.float32
    s123 = (g123 / np.sqrt(v123 + EPS)).astype(f32)            # (3, D)
    Wf = (W123 * s123[:, :, None]).astype(f32)                 # (3, D, C)
    bf = ((b123 - m123) * s123 + be123).astype(f32)            # (3, D)
    s4 = (g4 / np.sqrt(v4 + EPS)).astype(f32)                  # (C,)
    W4f = (W4 * s4[:, None]).astype(f32)                       # (C, D)
    b4f = ((b4 - m4) * s4 + be4).astype(f32)                   # (C,)

    # perm[j*128 + p] = 4p + j : row j*128+p of a device weight tensor
    # holds original channel 4p+j (matching the x packing).
    p_idx, j_idx = np.meshgrid(np.arange(128), np.arange(4), indexing="ij")
    perm = (4 * p_idx + j_idx).T.reshape(-1)                   # (512,)

    wqk = np.concatenate([Wf[0].T, Wf[1].T], axis=1)[perm]     # (C, 128)
    bqk = np.concatenate([bf[0], bf[1]])[:, None]              # (128, 1)
    w2a = np.ascontiguousarray(Wf[2].T[perm])                  # (C, D)
    b2a = bf[2][None, :]                                       # (1, D)
    w4t = np.ascontiguousarray(W4f.T[:, perm])                 # (D, C)
    b4v = b4f[perm][None, :]                                   # (1, C)
    msk = np.kron(np.eye(SUB, dtype=f32), np.ones((4, 4), f32))  # (128, 128)
    return dict(
        wqk=np.ascontiguousarray(wqk), bqk=np.ascontiguousarray(bqk),
        w2a=w2a, b2a=np.ascontiguousarray(b2a),
        w4t=w4t, b4v=np.ascontiguousarray(b4v), msk=msk,
    )


def _run(inputs, trace=False, **spmd_kwargs):
    from concourse.bass_utils import run_bass_kernel_spmd

    x = np.ascontiguousarray(np.asarray(inputs["x"], dtype=np.float32))
    params = prep_params(**{k: np.asarray(v, np.float64)
                            for k, v in inputs.items() if k != "x"})
    nc = build_program()
    in_maps = [
        {"x": x[i * NSH:(i + 1) * NSH], **params} for i in range(NCORES)
    ]
    res = run_bass_kernel_spmd(
        nc, in_maps, list(range(NCORES)), trace=trace, **spmd_kwargs
    )
    outs = np.concatenate(
        [np.asarray(res.results[i]["out"]) for i in range(NCORES)], axis=0
    )
    return outs, res


def kernel(**inputs):
    outs, _ = _run(inputs)
    return outs


# revision 18
# speedup vs baseline: 2.1520x; 1.7225x over previous
"""Trainium2 Bass kernel for the attention-gate block.

Math (per sample n, after folding BN into the convs):
  X     = x[n, :, ::2, ::2].reshape(C, 4)                 # C=512, L=4
  act_k = relu(Wk' @ X + bk')            k=0,1,2          # D=64 each
  S     = act0^T act1  (4x4);  P = softmax_rows(S)
  Z     = P @ act2^T  (4x64)
  Y     = W4' @ Z^T + b4'                                  # (512, 4)
  out[n,c,h,w] = x[n,c,h,w] + Y[c,h]                       # broadcast over w

Device mapping (per core, 256 samples, blocks of 64):
  - channel packing c = 4p + j (p = partition, j = 0..3): each
    (partition, sample) moves one 256B-contiguous run, so a block is ONE
    big DMA each way (loads on the sync HWDGE queue, stores on scalar's).
    Weights are permuted host-side to match, so compute is unchanged.
  - GEMM1 computes q and k over 4 contraction groups; v is computed
    directly transposed ([samples*4 parts, d]) by swapping matmul
    operands, with its bias folded in via a K=1 ones-row matmul.
  - attention scores for 32 samples at a time come from one [64]x[128,128]
    gram matmul whose block-diagonal 4x4 blocks are the real scores;
    masked exp (ACT + 0/1 block-diag mask on DVE); softmax denominators
    via a ones-column matmul; normalization deferred past the P@V matmul.
  - GEMM2 (BN folded) does the w-broadcast in its rhs AP (step-0 re-read
    of each z column) so the residual add runs on plain stride-1 APs.
"""

import os
import sys

for _p in ("/opt/trn_rl_repo",):
    if _p not in sys.path:
        sys.path.insert(0, _p)

import numpy as np

import concourse.mybir as mybir
from concourse import bacc, tile

EPS = 1e-5
N_TOTAL, C, D, HH, WW = 2048, 512, 64, 4, 4
NCORES = 8
NSH = N_TOTAL // NCORES  # 256 samples per core
BLK = int(os.environ.get("KBLK", "64"))  # samples per block
SUB = 32                 # samples per attention subchunk (4*SUB = 128 cols)
SHIFT = -34.0            # constant exp shift; cancels in the normalization
F32 = mybir.dt.float32

_PROG_CACHE = {}


def build_program(nsh=NSH, blk=BLK, reps=1):
    key = (nsh, blk, reps)
    if key in _PROG_CACHE:
        return _PROG_CACHE[key]

    nc = bacc.Bacc("TRN2", target_bir_lowering=False, debug=False)
    AF = mybir.ActivationFunctionType

    x_in = nc.dram_tensor("x", (nsh, C, HH, WW), F32, kind="ExternalInput")
    wqk = nc.dram_tensor("wqk", (C, 128), F32, kind="ExternalInput")
    bqk = nc.dram_tensor("bqk", (128, 1), F32, kind="ExternalInput")
    w2a = nc.dram_tensor("w2a", (C, D), F32, kind="ExternalInput")
    b2a = nc.dram_tensor("b2a", (1, D), F32, kind="ExternalInput")
    w4t = nc.dram_tensor("w4t", (D, C), F32, kind="ExternalInput")
    b4v = nc.dram_tensor("b4v", (1, C), F32, kind="ExternalInput")
    msk = nc.dram_tensor("msk", (128, 128), F32, kind="ExternalInput")
    out = nc.dram_tensor("out", (nsh, C, HH, WW), F32, kind="ExternalOutput")

    nblk = nsh // blk
    nsub = blk // SUB
    NF = 4 * blk  # free width of a full block of (n, l) columns

    with tile.TileContext(nc) as tc:
        with (
            tc.tile_pool(name="const", bufs=1) as cpool,
            tc.tile_pool(name="xp", bufs=(3 if blk >= 128 else 4)) as xpool,
            tc.tile_pool(name="work", bufs=4) as wpool,
            tc.tile_pool(name="att", bufs=6) as apool,
            tc.tile_pool(name="ps", bufs=6, space="PSUM") as pspool,
            tc.tile_pool(name="psy", bufs=2, space="PSUM") as pypool,
        ):
            wq_sb = cpool.tile([128, 4, D], F32)
            nc.sync.dma_start(
                wq_sb[:], wqk[:, 0:D].rearrange("(k p) d -> p k d", p=128))
            wk_sb = cpool.tile([128, 4, D], F32)
            nc.sync.dma_start(
                wk_sb[:], wqk[:, D:2 * D].rearrange("(k p) d -> p k d", p=128))
            bq_sb = cpool.tile([D, 1], F32)
            nc.sync.dma_start(bq_sb[:], bqk[0:D])
            bk_sb = cpool.tile([D, 1], F32)
            nc.sync.dma_start(bk_sb[:], bqk[D:2 * D])
            w2a_sb = cpool.tile([128, 4, D], F32)
            nc.sync.dma_start(w2a_sb[:], w2a[:].rearrange("(k p) d -> p k d", p=128))
            b2a_sb = cpool.tile([1, D], F32)
            nc.sync.dma_start(b2a_sb[:], b2a[:])
            w4t_sb = cpool.tile([D, 4, 128], F32)
            nc.sync.dma_start(w4t_sb[:], w4t[:].rearrange("d (k p) -> d k p", p=128))
            b4c_sb = cpool.tile([128, 4], F32)
            nc.sync.dma_start(
                b4c_sb[:], b4v[:].rearrange("x (j p) -> p (x j)", j=4))
            msk_sb = cpool.tile([128, 128], F32)
            nc.sync.dma_start(msk_sb[:], msk[:])
            ones_sb = cpool.tile([1, max(NF, 512)], F32)
            nc.vector.memset(ones_sb[:], 1.0)
            ones_col = cpool.tile([128, 1], F32)
            nc.vector.memset(ones_col[:], 1.0)
            shift_sb = cpool.tile([128, 1], F32)
            nc.vector.memset(shift_sb[:], SHIFT)

            # channel packing c = 4p + j: one DMA per block each way
            xv = x_in[:].rearrange("(b n) (p j) h w -> b p n (j h w)", j=4, n=blk)
            ov = out[:].rearrange("(b n) (p j) h w -> b p n (j h w)", j=4, n=blk)

            for b in [b for _ in range(reps) for b in range(nblk)]:
                x_t = xpool.tile([128, blk, 64], F32, tag="x")
                nc.sync.dma_start(x_t[:], xv[b])
                xtv = x_t[:].rearrange("p n (j h w) -> p n j h w", j=4, h=4)

                # gather the ::2,::2 columns -> [128, j, n, l] with l=(h',w')
                xr = wpool.tile([128, 4, blk, 4], F32, tag="xr")
                nc.vector.tensor_copy(
                    xr[:].rearrange("p j n (a c) -> p j n a c", a=2),
                    xtv[:, :, :, 0:4:2, 0:4:2].transpose([0, 2, 1, 3, 4]),
                )
                xrf = xr[:].rearrange("p j n l -> p j (n l)")

                # GEMM1 q and k: [c=512 contraction] -> psum [64, NF] each
                ps_q = pspool.tile([D, NF], F32, tag="ps")
                ps_k = pspool.tile([D, NF], F32, tag="ps")
                for j in range(4):
                    nc.tensor.matmul(
                        ps_q[:], lhsT=wq_sb[:, j], rhs=xrf[:, j],
                        start=(j == 0), stop=(j == 3),
                    )
                for j in range(4):
                    nc.tensor.matmul(
                        ps_k[:], lhsT=wk_sb[:, j], rhs=xrf[:, j],
                        start=(j == 0), stop=(j == 3),
                    )
                a_q = wpool.tile([D, NF], F32, tag="aq")
                nc.scalar.activation(a_q[:], ps_q[:], AF.Relu, bias=bq_sb[:])
                a_k = wpool.tile([D, NF], F32, tag="ak")
                nc.scalar.activation(a_k[:], ps_k[:], AF.Relu, bias=bk_sb[:])

                # phase 1: independent PE work for all subchunks
                ph_vt, ph_g = [], []
                for s in range(nsub):
                    cl = slice(s * 128, s * 128 + 128)
                    ps_vt = pspool.tile([128, D], F32, tag="ps")
                    for j in range(4):
                        nc.tensor.matmul(
                            ps_vt[:], lhsT=xrf[:, j, cl], rhs=w2a_sb[:, j],
                            start=(j == 0), stop=False,
                        )
                    nc.tensor.matmul(
                        ps_vt[:], lhsT=ones_sb[:, 0:128], rhs=b2a_sb[:],
                        start=False, stop=True,
                    )
                    ps_g = pspool.tile([128, 128], F32, tag="ps")
                    nc.tensor.matmul(
                        ps_g[:], lhsT=a_k[:, cl], rhs=a_q[:, cl],
                        start=True, stop=True,
                    )
                    ph_vt.append(ps_vt)
                    ph_g.append(ps_g)
                # phase 2: ACT/DVE consumers for all subchunks
                ph_a2t, ph_p0 = [], []
                for s in range(nsub):
                    a2t = apool.tile([128, D], F32, tag="a2t")
                    nc.scalar.activation(a2t[:], ph_vt[s][:], AF.Relu)
                    e_t = apool.tile([128, 128], F32, tag="e")
                    nc.scalar.activation(e_t[:], ph_g[s][:], AF.Exp,
                                         bias=shift_sb[:])
                    p0 = apool.tile([128, 128], F32, tag="p0")
                    nc.vector.tensor_mul(p0[:], e_t[:], msk_sb[:])
                    ph_a2t.append(a2t)
                    ph_p0.append(p0)
                # phase 3: dependent matmuls + normalization per subchunk
                z_subs = []
                for s in range(nsub):
                    a2t, p0 = ph_a2t[s], ph_p0[s]
                    ps_z = pspool.tile([D, 128], F32, tag="ps")
                    nc.tensor.matmul(
                        ps_z[:], lhsT=a2t[:], rhs=p0[:], start=True, stop=True,
                    )
                    ps_d = pspool.tile([1, 128], F32, tag="ps")
                    nc.tensor.matmul(
                        ps_d[:], lhsT=ones_col[:], rhs=p0[:],
                        start=True, stop=True,
                    )
                    r_sb = apool.tile([1, 128], F32, tag="r")
                    nc.vector.reciprocal(r_sb[:], ps_d[:])
                    ps_r = pspool.tile([D, 128], F32, tag="ps")
                    nc.tensor.matmul(
                        ps_r[:], lhsT=ones_sb[:, 0:D], rhs=r_sb[:],
                        start=True, stop=True,
                    )
                    r64_sb = apool.tile([D, 128], F32, tag="r64")
                    nc.scalar.activation(r64_sb[:], ps_r[:], AF.Copy)
                    z_t = apool.tile([D, 4 * SUB], F32, tag="z")
                    nc.vector.tensor_mul(z_t[:], ps_z[:], r64_sb[:])
                    z_subs.append(z_t)

                # GEMM2 + bias; the w-broadcast happens in the matmul rhs
                # (step-0 AP re-reads each z column 4x) so the residual
                # add runs on plain stride-1 APs at full DVE rate.
                nsb = SUB
                for j in range(4):
                    for h in range(nsub):
                        nsl = slice(h * nsb, (h + 1) * nsb)
                        zv = (
                            z_subs[h][:]
                            .rearrange("p (n l) -> p n l", l=4)
                            .unsqueeze(3)
                            .broadcast_to((D, nsb, 4, 4))
                        )
                        ps_y = pypool.tile([128, 16 * nsb], F32, tag="psy")
                        nc.tensor.matmul(
                            ps_y[:], lhsT=w4t_sb[:, j], rhs=zv[:],
                            start=True, stop=True,
                        )
                        nc.vector.scalar_tensor_tensor(
                            xtv[:, nsl, j],
                            ps_y[:].rearrange("p (n h w) -> p n h w", h=4, w=4),
                            b4c_sb[:, j:j + 1],
                            xtv[:, nsl, j],
                            op0=mybir.AluOpType.add,
                            op1=mybir.AluOpType.add,
                        )

                # store on the scalar HWDGE queue to overlap with loads
                nc.scalar.dma_start(ov[b], x_t[:])

    nc.compile()
    _PROG_CACHE[key] = nc
    return nc


def prep_params(W123, b123, g123, be123, m123, v123, W4, b4, g4, be4, m4, v4):
    """Fold BN into the convs; permute channels for the c=4p+j packing."""
    f32 = np.float32
    s123 = (g123 / np.sqrt(v123 + EPS)).astype(f32)            # (3, D)
    Wf = (W123 * s123[:, :, None]).astype(f32)                 # (3, D, C)
    bf = ((b123 - m123) * s123 + be123).astype(f32)            # (3, D)
    s4 = (g4 / np.sqrt(v4 + EPS)).astype(f32)                  # (C,)
    W4f = (W4 * s4[:, None]).astype(f32)                       # (C, D)
    b4f = ((b4 - m4) * s4 + be4).astype(f32)                   # (C,)

    # perm[j*128 + p] = 4p + j : row j*128+p of a device weight tensor
    # holds original channel 4p+j (matching the x packing).
    p_idx, j_idx = np.meshgrid(np.arange(128), np.arange(4), indexing="ij")
    perm = (4 * p_idx + j_idx).T.reshape(-1)                   # (512,)

    wqk = np.concatenate([Wf[0].T, Wf[1].T], axis=1)[perm]     # (C, 128)
    bqk = np.concatenate([bf[0], bf[1]])[:, None]              # (128, 1)
    w2a = np.ascontiguousarray(Wf[2].T[perm])                  # (C, D)
    b2a = bf[2][None, :]                                       # (1, D)
    w4t = np.ascontiguousarray(W4f.T[:, perm])                 # (D, C)
    b4v = b4f[perm][None, :]                                   # (1, C)
    msk = np.kron(np.eye(SUB, dtype=f32), np.ones((4, 4), f32))  # (128, 128)
    return dict(
        wqk=np.ascontiguousarray(wqk), bqk=np.ascontiguousarray(bqk),
        w2a=w2a, b2a=np.ascontiguousarray(b2a),
        w4t=w4t, b4v=np.ascontiguousarray(b4v), msk=msk,
    )


def _run(inputs, trace=False, **spmd_kwargs):
    from concourse.bass_utils import run_bass_kernel_spmd

    x = np.ascontiguousarray(np.asarray(inputs["x"], dtype=np.float32))
    params = prep_params(**{k: np.asarray(v, np.float64)
                            for k, v in inputs.items() if k != "x"})
    nc = build_program()
    in_maps = [
        {"x": x[i * NSH:(i + 1) * NSH], **params} for i in range(NCORES)
    ]
    res = run_bass_kernel_spmd(
        nc, in_maps, list(range(NCORES)), trace=trace, **spmd_kwargs
    )
    outs = np.concatenate(
        [np.asarray(res.results[i]["out"]) for i in range(NCORES)], axis=0
    )
    return outs, res


def kernel(**inputs):
    outs, _ = _run(inputs)
    return outs
